# revision 48
# baseline (speedup 1.0000x reference)
"""Trainium2 Bass kernel for the pairwise-distance masked log-sum loss.

Reference math (N=8192 points, E=49152 edges):
    dist[i,j] = |p_i - p_j|^2 + 1e-8
    mask      = (dist <= 0.25), edges (both directions) and diagonal zeroed
    loss      = sum(-log(dist) * mask)

Device strategy (8 NeuronCores, SPMD):
  * Points are sorted by x on the host.  Only pairs with |dx| < 0.5 can be
    inside the threshold, so each 128-row tile only needs its own blocks
    (intra-tile pairs) plus a forward window of sorted columns reaching
    x_max(tile) + 0.5.  Pairs outside the window contribute exactly 0.
    Forward windows count each inter-tile pair once; the host doubles that
    partial sum (dist is symmetric).  The 64 row tiles are dealt to the 8
    cores by snake order of window size, so every core runs an identical
    program on identically-shaped inputs (windows padded with far-away
    dummy points, which fall outside the threshold and contribute 0).
  * dist[i,j] = w_i . u_j with K=16 split-precision channels evaluated on
    the TensorE in float32r.  f32r is fp32 rounded to 11 explicit mantissa
    bits (measured round-to-nearest on hardware) at full bf16 PE rate; the
    host splits each coordinate c = ch + cl and |p|^2 = sqh + sql with
    rn11 so every channel is exactly representable and the product sum
    reconstructs |p_i - p_j|^2 + |p|^2-rounding (~1e-6) -- fp32-grade.
  * The diagonal 128x128 block has +10*I accumulated onto it by a second
    matmul (identity lhsT) so ln never sees the dist ~ 1e-8 diagonal.
  * ScalarE computes y = ln(dist) PSUM->SBUF (bf16), VectorE computes
    sum(y * (y <= ln .25)) per partition with the fused
    scalar_tensor_tensor accumulate.
  * Host: loss = -(S_diag + 2*S_windows) + 2*sum(ln dist) over the unique
    non-self edge pairs inside the threshold (the reference masks those
    out, the device sum includes them).
"""

import os

import numpy as np

N = 8192
NCORES = 8
ROW_TILE = 128
TILES = N // ROW_TILE  # 64
SLOTS = TILES // NCORES  # 8 row-tiles per core
USE_F16 = os.environ.get("KERNEL_F16", "1") == "1"
KCH = 18 if USE_F16 else 16  # split-precision channels
COL_CHUNK = 512  # one PSUM bank per matmul
GROUP_COLS = int(os.environ.get("KERNEL_GROUP_COLS", "1536"))  # ACT/DVE group
EPS = 1e-8
THR2 = 0.25
XWIN = 0.5
LN_THR = float(np.log(0.25))
DELTA = 6e-6  # positivity cushion folded into the u-side |p|^2 split

USE_FP32R = os.environ.get("KERNEL_FP32R", "1") == "1"
ACC_SLOTS = 64
POOL_SLOTS = int(os.environ.get("KERNEL_POOL_SLOTS", "0"))

LAST_RESULT = {}


def _rn(v: np.ndarray, bits: int) -> np.ndarray:
    """Round f32/f64 values to `bits` explicit mantissa bits (RN).  11 =
    the measured float32r grid; 10 = fp16 (normal range)."""
    v64 = np.asarray(v, dtype=np.float64)
    m, e = np.frexp(v64)
    q = np.ldexp(np.round(np.ldexp(m, bits + 1)) / (1 << (bits + 1)), e)
    return q.astype(np.float32)


def _build_channels(pts: np.ndarray):
    """w [KCH, n] and u [KCH, n] channel vectors on the input-dtype grid,
    such that sum_k w[k,i]*u[k,j] ~= |p_i - p_j|^2 (u side carries +DELTA
    so every distance, incl. the split-residual diagonal, stays positive
    for Ln)."""
    bits = 10 if USE_F16 else 11
    c = np.asarray(pts, dtype=np.float32)
    ch = _rn(c, bits)
    cl = _rn(c.astype(np.float64) - ch, bits)
    rep = ch.astype(np.float64) + cl  # represented points
    sq = (rep * rep).sum(axis=1)  # f64, exact-ish
    squ = sq + DELTA

    n = c.shape[0]
    w = np.empty((KCH, n), np.float32)
    u = np.empty((KCH, n), np.float32)
    for a in range(3):
        w[4 * a + 0] = -2.0 * ch[:, a]
        u[4 * a + 0] = ch[:, a]
        w[4 * a + 1] = -2.0 * ch[:, a]
        u[4 * a + 1] = cl[:, a]
        w[4 * a + 2] = -2.0 * cl[:, a]
        u[4 * a + 2] = ch[:, a]
        w[4 * a + 3] = -2.0 * cl[:, a]
        u[4 * a + 3] = cl[:, a]
    k = 12
    for val, side in ((sq, "w"), (squ, "u")):
        rem = val.copy()
        nsplit = 3 if USE_F16 else 2
        for _ in range(nsplit):
            hi = _rn(rem, bits)
            if side == "w":
                w[k] = hi
                u[k] = 1.0
            else:
                w[k] = 1.0
                u[k] = hi
            rem = rem - hi
            k += 1
    assert k == KCH
    return w, u


BANDS = 8


def _host_prep(pred_pos: np.ndarray):
    """Two-level sort (x-bands, y within band), per-tile geometric windows,
    snake balance; build per-core in_maps and program meta.

    Each row tile's window = [its own 128 columns] + every forward column
    that could be within the 0.5 threshold: same/later bands whose x-range
    is reachable, restricted to the tile's y-range +- 0.5.  Every unordered
    off-diagonal pair inside the threshold appears exactly once (own-tile
    lower triangle and diagonal are pushed out of the mask by the +10
    lower-tri matmul); the host doubles the device sum."""
    p = np.asarray(pred_pos, dtype=np.float32)
    per = N // BANDS
    xi = np.argsort(p[:, 0], kind="stable")
    psx = p[xi]
    order_parts = []
    band_x = []
    for b in range(BANDS):
        seg = np.arange(b * per, (b + 1) * per)
        band_x.append(
            (float(psx[seg, 0].min()), float(psx[seg, 0].max()))
        )
        yi = np.argsort(psx[seg, 1], kind="stable")
        order_parts.append(seg[yi])
    order = np.concatenate(order_parts)
    ps = psx[order]
    ys_band = [ps[b * per : (b + 1) * per, 1].astype(np.float64) for b in range(BANDS)]

    w, u = _build_channels(ps)

    CUSH = 1e-3
    tile_ranges = []  # per tile: list of (lo, hi) global column ranges
    for t in range(TILES):
        t0, t1 = t * ROW_TILE, (t + 1) * ROW_TILE
        b = t0 // per
        ya = float(ps[t0:t1, 1].min())
        yb = float(ps[t0:t1, 1].max())
        ranges = [(t0, t1)]  # own tile first (lower-tri masked on device)
        for b2 in range(b, BANDS):
            if band_x[b2][0] - band_x[b][1] >= XWIN - CUSH:
                break
            lo = int(np.searchsorted(ys_band[b2], ya - XWIN - CUSH))
            hi = int(np.searchsorted(ys_band[b2], yb + XWIN + CUSH, side="right"))
            lo += b2 * per
            hi += b2 * per
            if b2 == b:
                lo = max(lo, t1)
            if hi > lo:
                ranges.append((lo, hi))
        tile_ranges.append(ranges)

    widths = [sum(hi - lo for lo, hi in r) for r in tile_ranges]

    # snake-deal tiles (desc width) to cores; slot s width = max in band
    rank = sorted(range(TILES), key=lambda t: -widths[t])
    assign = [[None] * SLOTS for _ in range(NCORES)]
    for s in range(SLOTS):
        band = rank[s * NCORES : (s + 1) * NCORES]
        cores = range(NCORES) if s % 2 == 0 else range(NCORES - 1, -1, -1)
        for t, c in zip(band, cores):
            assign[c][s] = t
    slot_w = []
    for s in range(SLOTS):
        wmax = max(widths[assign[c][s]] for c in range(NCORES))
        slot_w.append(int(np.ceil(wmax / ROW_TILE)) * ROW_TILE)

    # process the largest slot first so the kernel tail (last group's
    # ACT/DVE chain) is as short as possible
    perm = sorted(range(SLOTS), key=lambda s: -slot_w[s])
    slot_w = [slot_w[s] for s in perm]
    assign = [[assign[c][perm[s]] for s in range(SLOTS)] for c in range(NCORES)]

    # dummy far-away point channels (outside any threshold window)
    _, ud = _build_channels(np.array([[100.0, 0.0, 0.0]], np.float32))

    # Striped [128, X] input layout: stripe q (partitions [32q, 32q+KCH),
    # q in {0,1,2} -- matmul base partitions may only be 0/32/64) carries
    # a few slots' data [rowsW_s | win_s]..., so ONE wide DMA runs at high
    # port bandwidth.  Greedy assignment keeps stripe lengths balanced.
    stripe_of = {}
    col_of = {}
    stripe_len = [0, 0, 0]
    for s in range(SLOTS):  # slots already sorted by descending width
        q = min(range(3), key=lambda i: stripe_len[i])
        stripe_of[s] = q
        col_of[s] = (stripe_len[q], stripe_len[q] + ROW_TILE)
        stripe_len[q] += ROW_TILE + slot_w[s]
    X = max(stripe_len)

    in_maps = []
    ymask = np.tril(np.full((128, 128), 50.0, np.float32))
    np_in_dt = np.float16 if USE_F16 else np.float32
    for c in range(NCORES):
        inp = np.zeros((128, X + 128), np_in_dt)
        inp[:, X:] = ymask
        for s in range(SLOTS):
            t = assign[c][s]
            q = stripe_of[s]
            rw_off, win_off = col_of[s]
            r0 = t * ROW_TILE
            inp[32 * q : 32 * q + KCH, rw_off : rw_off + ROW_TILE] = w[
                :, r0 : r0 + ROW_TILE
            ]
            o = win_off
            for lo, hi in tile_ranges[t]:
                inp[32 * q : 32 * q + KCH, o : o + hi - lo] = u[:, lo:hi]
                o += hi - lo
            if o < win_off + slot_w[s]:
                inp[32 * q : 32 * q + KCH, o : win_off + slot_w[s]] = ud
        in_maps.append({"inp": np.ascontiguousarray(inp)})

    meta = {
        "slot_w": slot_w,
        "width": X + 128,
        "ymask_off": X,
        "stripe_of": stripe_of,
        "col_of": col_of,
    }
    return in_maps, meta


def _edge_correction(pred_pos: np.ndarray, edges: np.ndarray) -> float:
    """sum of ln(dist) over unique unordered non-self edge pairs inside the
    threshold (each such pair appears exactly twice in the device sum)."""
    p = np.asarray(pred_pos, dtype=np.float32)
    e = np.asarray(edges, dtype=np.int64)
    e = e[e[:, 0] != e[:, 1]]
    e = np.sort(e, axis=1)
    e = np.unique(e, axis=0)
    d = p[e[:, 0]] - p[e[:, 1]]
    dist = (d * d).sum(axis=1, dtype=np.float32) + np.float32(EPS)
    m = dist <= np.float32(THR2)
    return float(np.log(dist[m].astype(np.float64)).sum())


def _build_program(meta):
    import concourse.bass as bass
    import concourse.tile as tile
    from concourse import mybir
    from contextlib import ExitStack

    f32 = mybir.dt.float32
    bf16 = mybir.dt.bfloat16
    if USE_F16:
        in_dt = mybir.dt.float16
    elif USE_FP32R:
        in_dt = mybir.dt.float32r
    else:
        in_dt = f32

    slot_w = meta["slot_w"]
    width = meta["width"]

    nc = bass.Bass("TRN2", target_bir_lowering=False, debug=False, num_devices=NCORES)
    inp_d = nc.dram_tensor("inp", [128, width], in_dt, kind="ExternalInput").ap()
    acc_d = nc.dram_tensor("acc", [128, ACC_SLOTS], f32, kind="ExternalOutput").ap()

    # how many ACT/DVE groups in total (for y-buffer count: no slot reuse)
    n_groups = sum((wl + GROUP_COLS - 1) // GROUP_COLS for wl in slot_w)
    assert n_groups <= ACC_SLOTS

    with tile.TileContext(nc) as tc, ExitStack() as ctx:
        singles = ctx.enter_context(tc.tile_pool(name="singles", bufs=1))
        psum_bufs = (8 * 512) // GROUP_COLS  # use all 8 PSUM banks
        psums = ctx.enter_context(
            tc.tile_pool(name="psums", bufs=psum_bufs, space="PSUM")
        )
        ys = ctx.enter_context(tc.tile_pool(name="ys", bufs=n_groups))
        scraps = ctx.enter_context(tc.tile_pool(name="scraps", bufs=2))

        inp_s = singles.tile([128, width], in_dt)
        nc.sync.dma_start(out=inp_s, in_=inp_d)
        ymask_s = inp_s[:, meta["ymask_off"] : meta["ymask_off"] + 128]
        acc_s = singles.tile([128, ACC_SLOTS], f32)
        acc_p = singles.tile([128, ACC_SLOTS], f32)
        pool_accs = []

        # prime DVE's view of the input DMA queue so the per-slot ymask
        # adds don't need a second (DMA) wait besides their ACT wait
        prime_t = singles.tile([128, 1], in_dt)
        nc.vector.tensor_copy(out=prime_t, in_=ymask_s[:, 0:1])

        def reduce_group(psum_t, cols, acc_idx, mask_own, eng):
            y_t = ys.tile([128, GROUP_COLS], bf16, tag="y")
            nc.scalar.activation(
                out=y_t[:, :cols],
                in_=psum_t[:, :cols],
                func=mybir.ActivationFunctionType.Ln,
            )
            if mask_own:
                # +50 on the own-tile lower triangle and diagonal pushes
                # those y values far above the threshold -> excluded
                eng.tensor_tensor(
                    out=y_t[:, :ROW_TILE],
                    in0=y_t[:, :ROW_TILE],
                    in1=ymask_s,
                    op=mybir.AluOpType.add,
                )
            scrap_t = scraps.tile([128, GROUP_COLS], bf16, tag="scrap")
            acc_dst = acc_s if eng is nc.vector else acc_p
            if eng is not nc.vector:
                pool_accs.append(acc_idx)
            eng.scalar_tensor_tensor(
                out=scrap_t[:, :cols],
                in0=y_t[:, :cols],
                scalar=LN_THR,
                in1=y_t[:, :cols],
                op0=mybir.AluOpType.is_le,
                op1=mybir.AluOpType.mult,
                accum_out=acc_dst[:, acc_idx : acc_idx + 1],
            )

        # Each slot's window = [own 128 cols | forward cols]; the own-tile
        # lower triangle and diagonal are pushed out of the ln-threshold
        # mask by the +50*tril ymask add on y.
        stripe_of = meta["stripe_of"]
        col_of = meta["col_of"]
        acc_idx = 0
        for s in range(SLOTS):
            q = stripe_of[s]
            p0 = 32 * q
            rw_off, win_off = col_of[s]
            lhsT = inp_s[p0 : p0 + KCH, rw_off : rw_off + ROW_TILE]
            wl = slot_w[s]
            done = 0
            while done < wl:
                cols = min(GROUP_COLS, wl - done)
                psum_t = psums.tile([128, GROUP_COLS], f32, tag="ps")
                for k0 in range(0, cols, COL_CHUNK):
                    kw = min(COL_CHUNK, cols - k0)
                    c0 = win_off + done + k0
                    nc.tensor.matmul(
                        out=psum_t[:, k0 : k0 + kw],
                        lhsT=lhsT,
                        rhs=inp_s[p0 : p0 + KCH, c0 : c0 + kw],
                        start=True,
                        stop=True,
                    )
                eng = nc.gpsimd if s >= SLOTS - POOL_SLOTS else nc.vector
                reduce_group(psum_t, cols, acc_idx, done == 0, eng)
                acc_idx += 1
                done += cols
        assert acc_idx <= ACC_SLOTS
        meta["n_groups_used"] = acc_idx

        if pool_accs:
            # GpSimd accumulates into its own tile; one DVE copy funnels it
            # into acc_s so the final DMA carries only the DVE wait
            lo, hi = min(pool_accs), max(pool_accs) + 1
            nc.vector.tensor_copy(
                out=acc_s[:, lo:hi], in_=acc_p[:, lo:hi]
            )

        # only the written accumulator columns -- the rest of the SBUF
        # tile is uninitialized garbage
        nc.sync.dma_start(out=acc_d[:, :acc_idx], in_=acc_s[:, :acc_idx])

    _strip_self_waits(nc, mybir)
    return nc


_SELF_WAIT_OPCODES = {
    "InstMatmult",
    "InstTensorScalarPtr",
    "InstActivation",
    "InstTensorTensor",
    "InstTensorReduce",
    "InstTensorCopy",
    "InstMemset",
}
_ENGINE_SEM_PREFIX = {
    "PE": "PE_",
    "ACT": "Activation_",
    "DVE": "DVE_",
    "POOL": "Pool_",
    "SP": "SP_",
}


def _strip_self_waits(nc, mybir):
    """Walrus caps sync-wait commands per instruction (1 for PE/DVE compute
    structs).  Make every instruction single-wait:
      * compute ops: drop same-engine self-waits (in-order engines make
        them vacuous);
      * DMACopy: drop cross-queue DMA-ordering waits (all SBUF regions
        involved here are disjoint);
      * Drain (kernel tail): split into a chain of single-wait drains;
      * anything else left with >1 wait: fail loudly (do NOT guess).
    """
    for fn in nc.m.functions:
        for bb in fn.blocks:
            for inst in bb.instructions:
                si = inst.sync_info
                if si is None or not si.on_wait or len(si.on_wait) < 2:
                    continue
                tname = type(inst).__name__
                waits = list(si.on_wait)
                if tname == "InstDMACopy":
                    keep = [
                        w
                        for w in waits
                        if not w.ant_name.startswith(("DMAHW", "DMASW"))
                    ]
                elif tname in _SELF_WAIT_OPCODES:
                    eng = getattr(inst.engine, "name", str(inst.engine))
                    prefix = None
                    for k, v in _ENGINE_SEM_PREFIX.items():
                        if k in str(eng).upper():
                            prefix = v
                            break
                    if prefix is None:
                        continue
                    keep = [w for w in waits if not w.ant_name.startswith(prefix)]
                else:
                    continue
                if keep and len(keep) < len(waits):
                    inst.sync_info = mybir.SyncInfo(
                        on_wait=keep, on_update=si.on_update
                    )

    split_id = 0
    for fn in nc.m.functions:
        for bb in fn.blocks:
            idx = 0
            insts = bb.instructions
            while idx < len(insts):
                inst = insts[idx]
                si = inst.sync_info
                if (
                    type(inst).__name__ == "InstDrain"
                    and si is not None
                    and si.on_wait
                    and len(si.on_wait) > 1
                ):
                    waits = list(si.on_wait)
                    inst.sync_info = mybir.SyncInfo(
                        on_wait=[waits[-1]], on_update=si.on_update
                    )
                    for w in waits[:-1]:
                        nd = mybir.InstDrain(
                            name=f"I-drainsplit-{split_id}",
                            ins=[],
                            outs=[],
                            bass_is_fusable=False,
                        )
                        split_id += 1
                        nd.engine = inst.engine
                        nd.sync_info = mybir.SyncInfo(on_wait=[w], on_update=[])
                        insts.insert(idx, nd)
                        idx += 1
                idx += 1

    for fn in nc.m.functions:
        for bb in fn.blocks:
            for inst in bb.instructions:
                si = inst.sync_info
                if si is not None and si.on_wait and len(si.on_wait) > 1:
                    if type(inst).__name__ in ("InstEventSemaphore",):
                        continue
                    raise RuntimeError(
                        f"{inst.name} ({type(inst).__name__}) still has "
                        f"{len(si.on_wait)} waits: "
                        f"{[w.ant_name for w in si.on_wait]}"
                    )


def _finalize(results, pred_pos, edges, n_used) -> np.float32:
    # every unordered off-diagonal pair inside the threshold appears exactly
    # once in the device sum -> double it; edge pairs likewise.
    s_all = 0.0
    for r in results:
        s_all += r["acc"][:, :n_used].astype(np.float64).sum()
    corr = _edge_correction(pred_pos, edges)
    return np.float32(-2.0 * s_all + 2.0 * corr)


def kernel(pred_pos: np.ndarray, edges: np.ndarray) -> np.ndarray:
    from concourse.bass_utils import run_bass_kernel_spmd

    in_maps, meta = _host_prep(pred_pos)
    nc = _build_program(meta)
    trace = os.environ.get("KERNEL_TRACE", "0") == "1"
    trace_cores = None
    if os.environ.get("KERNEL_TRACE_ALL", "0") == "1":
        trace_cores = list(range(NCORES))
    res = run_bass_kernel_spmd(
        nc,
        in_maps,
        core_ids=list(range(NCORES)),
        trace=trace,
        trace_cores=trace_cores,
    )
    LAST_RESULT["exec_time_ns"] = res.exec_time_ns
    LAST_RESULT["trace"] = res.instructions_and_trace
    LAST_RESULT["meta"] = meta

    return _finalize(res.results, pred_pos, edges, meta["n_groups_used"])
